# revision 29
# baseline (speedup 1.0000x reference)
"""Trainium2 Bass kernel for nn_DensityLoss (raw Block mode, SPMD x8, replicated).

Math
----
reference(centers, features, labels) depends only on centers [C=4096, D=256]
(features unused; labels only via N=len(labels)=262144, a constant):

    sq_i  = ||c_i||^2;  m = sum_i c_i;  S = sum sq;  proj_i = c_i . m
    n_i   = C*sq_i + S - 2*proj_i          (center_dist_i = n_i/(C-1); diag==0)
    sum n   = 2*C*S - 2*m.m                (h := m.m - 256*C*S' = -sum n/2)
    sum n^2 = C^2 q + 3C S^2 + 4 m'Sigma m - 4C (w.m) - 4S (m.m)
        q = sum sq^2, w = sum sq_i c_i, Sigma = X'X, S' = S/256
    result = k*h/d',  k = -2(C-1)^2 E/(C N),  d' = E*(sum n^2 - (sum n)^2/C)

Implementation (v3; the old kernel's GPSIMD f32->bf16 cast was its 27us
bottleneck; DMA 4MiB is the 11.1us floor):
  - DMA: p-major contiguous chunks ([4]*7+[2,2] row-tiles; small final chunks
    shorten the post-DMA critical path).
  - DVE: X cast f32->bf16 of all chunks (~0.42 cyc/elem), 14 of 32 per-row
    sq via one scalar_tensor_tensor each ((x/256)*x, accum_out = sq'=sq/256),
    per-chunk bf16 cast of the sq' column, f32 S'/q' partials (q' prescaled
    by 2^20 exactly).
  - GPSIMD: idle.  Its ~840ns/tile cast rate stalls PE's in-order tile
    consumption (paired A/B: gps=0 beats gps=1 by ~3us, gps=2 by ~5us).
  - ACT: the other 18 sq tiles via activation(Square(x/16), accum_out).
  - PE:  bf16 Gram of A=[X|1|sq'] as two full-width row blocks
    psA=G[0:128,0:258], psB=G[128:256,0:258]; tail: v0=B00 m0+B01 m1,
    v1=B10 m0+B11 m1 (4 N=1 matmuls, Gram symmetry), psS = 1'e (S',2^20 q'
    cross-partition sums), psQ = m0'[m0 w0 v0] + m1'[m1 w1 v1] = [mm, wm,
    m'Sigma m] -- so the serial scalar chain is 13 DVE ops on partition 0.
  - S'/q' stay f32 end-to-end (catastrophic var cancellation; Sigma/m/w only
    enter small terms, bf16 there is ~1e-7 on the result).
Centers replicated to all 8 cores (8-core AllReduce has a ~20us floor; the
whole kernel is shorter).

Modes: "serial" chains rounds through s_res (honest end-to-end latency per
round; kernel() runs rounds=1); "p1" chains rounds but runs the tail once
(isolates tail cost vs serial); "pipe" overlaps rounds (steady-state rate).
"""

import numpy as np

C, D = 4096, 256
N_LABELS = 262144
P = 128
NT = C // P            # 32 row tiles
W = D + 2              # 258: [X | ones | sq']
WP = 264               # padded SBUF row stride (1056 B, 32B-aligned)
GW = 260               # Ga/Gb width: [G row block (258) | pad | v col @258]
CHUNK_SIZES = [4, 4, 4, 4, 4, 4, 4, 2, 2]
ACT_TILES = [3, 2, 2, 2, 3, 2, 2, 1, 1]   # leading sq tiles per chunk on ACT
GPS_CHUNKS = 0                            # GPSIMD idle: its slow cast delayed PE's in-order consumption
DUAL_RING = True                          # odd chunks' DMA issued from the ACT HWDGE ring
N_CORES = 8

_CACHE = {}


def _build_nc(rounds=1, mode="serial"):
    import concourse.bass as bass
    from concourse import mybir

    NCH = len(CHUNK_SIZES)
    CHUNK_OFFS = [sum(CHUNK_SIZES[:i]) for i in range(NCH)]
    assert sum(CHUNK_SIZES) == NT and len(ACT_TILES) == NCH
    assert mode in ("serial", "p1", "pipe")
    serial = mode == "serial"
    f32 = mybir.dt.float32
    bf16 = mybir.dt.bfloat16
    Alu = mybir.AluOpType
    Act = mybir.ActivationFunctionType

    nc = bass.Bass()
    x_ext = nc.declare_dram_parameter("centers", [C, D], f32, isOutput=False)
    out_ext = nc.declare_dram_parameter("out", [1, 1], f32, isOutput=True)

    xv = x_ext[:, :].rearrange("(p t) d -> p t d", p=P)   # [128, 32, 256]

    from contextlib import ExitStack

    with ExitStack() as ctx:
        en = ctx.enter_context
        xh = en(nc.sbuf_tensor([P, NT, WP], f32))
        xhb = en(nc.sbuf_tensor([P, NT, WP], bf16))
        scr = en(nc.sbuf_tensor([P, D], f32))      # ACT Square main-out sink
        scrv = en(nc.sbuf_tensor([P, D], f32))     # DVE STT main-out sink
        sq2 = en(nc.sbuf_tensor([P, NT], f32))
        zc = en(nc.sbuf_tensor([P, 1], f32))
        ones_col = en(nc.sbuf_tensor([P, 1], f32))
        Ga = en(nc.sbuf_tensor([P, GW], f32))
        Gb = en(nc.sbuf_tensor([P, GW], f32))
        e = en(nc.sbuf_tensor([P, 2], f32))
        sc = en(nc.sbuf_tensor([1, 32], f32))
        res = en(nc.sbuf_tensor([1, 1], f32))
        psA = en(nc.psum_tensor([P, W], f32))
        psB = en(nc.psum_tensor([P, W], f32))
        psV0 = en(nc.psum_tensor([P, 1], f32))
        psV1 = en(nc.psum_tensor([P, 1], f32))
        psS = en(nc.psum_tensor([1, 2], f32))
        psQ = en(nc.psum_tensor([1, 3], f32))
        s_dma = [en(nc.semaphore(f"s_dma{i}")) for i in range(NCH)]
        s_pre = en(nc.semaphore("s_pre"))
        s_xb = en(nc.semaphore("s_xb"))
        s_gx = en(nc.semaphore("s_gx"))
        s_sqa = en(nc.semaphore("s_sqa"))
        s_sqd = en(nc.semaphore("s_sqd"))
        s_pe = en(nc.semaphore("s_pe"))
        s_mm = en(nc.semaphore("s_mm"))
        s_cpa = en(nc.semaphore("s_cpa"))
        s_cpb = en(nc.semaphore("s_cpb"))
        s_mv = en(nc.semaphore("s_mv"))
        s_vc = en(nc.semaphore("s_vc"))
        s_q = en(nc.semaphore("s_q"))
        s_er = en(nc.semaphore("s_er"))
        s_res = en(nc.semaphore("s_res"))
        s_out = en(nc.semaphore("s_out"))
        block = en(nc.Block())

        m0 = Ga[:, D:D + 1]
        m1 = Gb[:, D:D + 1]
        sqv = xh[:, :, D + 1]                      # [128, 32] sq' column
        n_tail = rounds if serial else 1
        Cf = float(C)
        E = 2.0 ** -20

        dual = DUAL_RING and mode != "pipe"

        @block.sync
        def _(sync):
            for r in range(rounds):
                for ci in range(NCH):
                    if dual and ci % 2 == 1:
                        continue
                    lo = CHUNK_OFFS[ci]
                    hi = lo + CHUNK_SIZES[ci]
                    if ci == 0 and r > 0:
                        if serial:
                            sync.wait_ge(s_res, r)
                        else:
                            sync.wait_ge(s_er, r)
                            sync.wait_ge(s_pe, NCH * r)
                            sync.wait_ge(s_gx, GPS_CHUNKS * r)
                    if mode == "pipe" and r > 0:
                        # WAR: round r overwrites chunk ci once round r-1's
                        # readers (DVE cast+sq'cast, ACT sq, GPS cast, PE)
                        # are done
                        sync.wait_ge(s_sqd, NCH * (r - 1) + ci + 1)
                        sync.wait_ge(s_pe, NCH * (r - 1) + ci + 1)
                        if ci < GPS_CHUNKS:
                            sync.wait_ge(s_gx, GPS_CHUNKS * (r - 1) + ci + 1)
                    sync.dma_start(
                        out=xh[:, lo:hi, 0:D], in_=xv[:, lo:hi, :],
                    ).then_inc(s_dma[ci], 16)
            sync.wait_ge(s_res, n_tail)
            sync.dma_start(out=out_ext[:, :], in_=res[:, :]).then_inc(s_out, 16)
            sync.wait_ge(s_out, 16)

        @block.vector
        def _(vector):
            # preamble (cols disjoint from DMA'd cols 0:256)
            vector.memset(xhb[:, :, D:D + 1], 1.0)
            vector.memset(zc[:, :], 0.0)
            nc.vector.memset(ones_col[:, :], 1.0).then_inc(s_pre, 1)
            TS = nc.vector.tensor_scalar
            TT = nc.vector.tensor_tensor
            STT = nc.vector.scalar_tensor_tensor
            for r in range(rounds):
                for ci in range(NCH):
                    lo = CHUNK_OFFS[ci]
                    hi = lo + CHUNK_SIZES[ci]
                    vector.wait_ge(s_dma[ci], 16 * (r + 1))
                    if ci >= GPS_CHUNKS:
                        # X cast for the whole chunk (GPS casts chunks 0-1)
                        nc.vector.tensor_copy(
                            out=xhb[:, lo:hi, 0:D], in_=xh[:, lo:hi, 0:D],
                        ).then_inc(s_xb, 1)
                    # sq via (x/256)*x with accum -> sq' = sq/256 (f32)
                    for t in range(lo + ACT_TILES[ci], hi):
                        STT(scrv[:, :], xh[:, t, 0:D], 1.0 / 256.0,
                            xh[:, t, 0:D], op0=Alu.mult, op1=Alu.mult,
                            accum_out=xh[:, t, D + 1:D + 2])
                    vector.wait_ge(s_sqa, NCH * r + ci + 1)
                    vector.drain()
                    nc.vector.tensor_copy(
                        out=xhb[:, lo:hi, D + 1:D + 2],
                        in_=xh[:, lo:hi, D + 1:D + 2],
                    ).then_inc(s_sqd, 1)
                # S' and 2^20*q' partials (f32, precision-critical; 2^20 exact)
                nc.vector.tensor_reduce(e[:, 0:1], sqv,
                                        axis=mybir.AxisListType.X, op=Alu.add)
                STT(sq2[:, :], sqv, 2.0 ** 20, sqv, op0=Alu.mult, op1=Alu.mult)
                vector.drain()
                nc.vector.tensor_reduce(e[:, 1:2], sq2[:, :],
                                        axis=mybir.AxisListType.X,
                                        op=Alu.add).then_inc(s_er, 1)
                if not serial and r < rounds - 1:
                    continue
                # ---- tail ----
                rr = (r + 1) if serial else 1
                vector.wait_ge(s_mm, rr)
                nc.vector.tensor_copy(Gb[:, 0:W], psB[:, :]).then_inc(s_cpb, 1)
                vector.wait_ge(s_mv, rr)
                nc.vector.tensor_copy(Ga[:, W:W + 1], psV0[:, :])
                nc.vector.tensor_copy(Gb[:, W:W + 1],
                                      psV1[:, :]).then_inc(s_vc, 1)
                vector.wait_ge(s_q, rr)
                # scalar chain on partition 0.  h^2 in T6 is expanded exactly
                # (h = mm - 2^20 S'; pow2/integer coefficients), merging into
                # T2,T5: every d' term is a direct product of stage-0 values:
                #   d' = T1 - 256 S'^2 + 4E mSm - 4 wm + 2^-10 S' mm
                #        - 2^-30 mm^2
                # sc: 0 mm | 1 wm | 2 mSm | 18 S' | 19 T1=2^20 q' | 8 h
                #     | 20..24 T-terms | 11 d' | 12 1/d'

                def s(i):
                    return sc[:, i:i + 1]

                nc.vector.tensor_copy(sc[:, 0:3], psQ[0:1, 0:3])
                nc.vector.tensor_copy(sc[:, 18:20], psS[0:1, 0:2])
                vector.drain()
                STT(s(20), s(18), -256.0, s(18), op0=Alu.mult, op1=Alu.mult)
                STT(s(21), s(18), 2.0 ** -10, s(0), op0=Alu.mult,
                    op1=Alu.mult)
                STT(s(22), s(0), -(2.0 ** -30), s(0), op0=Alu.mult,
                    op1=Alu.mult)
                TS(s(23), s(2), 4.0 * E, None, op0=Alu.mult)
                TS(s(24), s(1), -4.0, None, op0=Alu.mult)
                STT(s(8), s(18), -256.0 * Cf, s(0), op0=Alu.mult, op1=Alu.add)
                vector.drain()
                nc.vector.tensor_reduce(s(11), sc[:, 19:25],
                                        axis=mybir.AxisListType.X, op=Alu.add)
                vector.drain()
                nc.vector.reciprocal(s(12), s(11))
                vector.drain()
                k = -2.0 * (Cf - 1.0) ** 2 / (Cf * float(N_LABELS)) * E
                STT(res[:, :], s(8), k, s(12), op0=Alu.mult,
                    op1=Alu.mult).then_inc(s_res, 1)

        @block.scalar
        def _(scalar):
            scalar.wait_ge(s_pre, 1)
            for r in range(rounds):
                if dual:
                    # odd chunks' DMA from the second HWDGE ring, issued
                    # before this round's squares
                    for ci in range(1, NCH, 2):
                        lo = CHUNK_OFFS[ci]
                        hi = lo + CHUNK_SIZES[ci]
                        if ci == 1 and r > 0:
                            if serial:
                                scalar.wait_ge(s_res, r)
                            else:
                                scalar.wait_ge(s_er, r)
                                scalar.wait_ge(s_pe, NCH * r)
                        nc.scalar.dma_start(
                            out=xh[:, lo:hi, 0:D], in_=xv[:, lo:hi, :],
                        ).then_inc(s_dma[ci], 16)
                if mode == "pipe" and r > 0:
                    # WAR: round r's accum writes to the sq' col race round
                    # r-1's e reductions on DVE
                    scalar.wait_ge(s_er, r)
                for ci in range(NCH):
                    lo = CHUNK_OFFS[ci]
                    if ACT_TILES[ci] == 0:
                        # keep s_sqa chunk-ordered; fires early, off the path
                        nc.scalar.copy(out=scr[:, 0:1],
                                       in_=zc[:, :]).then_inc(s_sqa, 1)
                        continue
                    scalar.wait_ge(s_dma[ci], 16 * (r + 1))
                    op = None
                    for t in range(lo, lo + ACT_TILES[ci]):
                        op = nc.scalar.activation(
                            out=scr[:, :], in_=xh[:, t, 0:D], func=Act.Square,
                            bias=zc[:, :], scale=0.0625,
                            accum_out=xh[:, t, D + 1:D + 2])
                    op.then_inc(s_sqa, 1)
                if serial or r == rounds - 1:
                    scalar.wait_ge(s_mm, (r + 1) if serial else 1)
                    nc.scalar.copy(Ga[:, 0:W], psA[:, :]).then_inc(s_cpa, 1)

        @block.gpsimd
        def _(gpsimd):
            for r in range(rounds):
                for ci in range(GPS_CHUNKS):
                    lo = CHUNK_OFFS[ci]
                    hi = lo + CHUNK_SIZES[ci]
                    gpsimd.wait_ge(s_dma[ci], 16 * (r + 1))
                    nc.gpsimd.tensor_copy(
                        out=xhb[:, lo:hi, 0:D], in_=xh[:, lo:hi, 0:D],
                    ).then_inc(s_gx, 1)

        @block.tensor
        def _(tensor):
            tensor.wait_ge(s_pre, 1)
            for r in range(rounds):
                for t in range(NT):
                    ci = next(i for i in range(NCH)
                              if CHUNK_OFFS[i] <= t < CHUNK_OFFS[i]
                              + CHUNK_SIZES[i])
                    if ci < GPS_CHUNKS:
                        tensor.wait_ge(s_gx, GPS_CHUNKS * r + ci + 1)
                    else:
                        tensor.wait_ge(s_xb,
                                       (NCH - GPS_CHUNKS) * r
                                       + ci - GPS_CHUNKS + 1)
                    tensor.wait_ge(s_sqd, NCH * r + ci + 1)
                    first = (t == 0 and (serial or r == 0))
                    last = (t == NT - 1 and (serial or r == rounds - 1))
                    ma = nc.tensor.matmul(psA[:, :], xhb[:, t, 0:P],
                                          xhb[:, t, 0:W], start=first,
                                          stop=last)
                    mm = nc.tensor.matmul(psB[:, :], xhb[:, t, P:D],
                                          xhb[:, t, 0:W], start=first,
                                          stop=last)
                    if t == CHUNK_OFFS[ci] + CHUNK_SIZES[ci] - 1:
                        ma.then_inc(s_pe, 1)
                    if last:
                        mm.then_inc(s_mm, 1)
                if serial or r == rounds - 1:
                    rr = (r + 1) if serial else 1
                    tensor.wait_ge(s_er, rr)
                    nc.tensor.matmul(psS[:, :], ones_col[:, :], e[:, :],
                                     start=True, stop=True)
                    tensor.wait_ge(s_cpa, rr)
                    tensor.wait_ge(s_cpb, rr)
                    # v0 = B00 m0 + B01 m1; v1 = B10 m0 + B11 m1 (symmetry)
                    nc.tensor.matmul(psV0[:, :], Ga[:, 0:P], m0,
                                     start=True, stop=False)
                    nc.tensor.matmul(psV0[:, :], Gb[:, 0:P], m1,
                                     start=False, stop=True)
                    nc.tensor.matmul(psV1[:, :], Ga[:, P:D], m0,
                                     start=True, stop=False)
                    nc.tensor.matmul(psV1[:, :], Gb[:, P:D], m1,
                                     start=False, stop=True).then_inc(s_mv, 1)
                    tensor.wait_ge(s_vc, rr)
                    # psQ = m0'[m0 w0 v0] + m1'[m1 w1 v1] = [mm, wm, mSm]
                    nc.tensor.matmul(psQ[:, :], m0, Ga[:, D:D + 3],
                                     start=True, stop=False)
                    nc.tensor.matmul(psQ[:, :], m1, Gb[:, D:D + 3],
                                     start=False, stop=True).then_inc(s_q, 1)

    return nc


def _get_nc(rounds=1, mode="serial"):
    key = ("nc", rounds, mode)
    if key not in _CACHE:
        _CACHE[key] = _build_nc(rounds, mode)
    return _CACHE[key]


def run(centers: np.ndarray, trace: bool = False):
    """Run the SPMD kernel on cores 0-7; returns (scalar ndarray, results)."""
    from concourse.bass_utils import run_bass_kernel_spmd

    nc = _get_nc()
    x = np.ascontiguousarray(np.asarray(centers, dtype=np.float32))
    in_maps = [{"centers": x} for _ in range(N_CORES)]
    r = run_bass_kernel_spmd(nc, in_maps, core_ids=list(range(N_CORES)),
                             trace=trace)
    # all 8 cores compute the same scalar; median guards a flaky core
    vals = np.array([np.asarray(res["out"]).reshape(()) for res in r.results],
                    dtype=np.float32)
    out = np.median(vals).astype(np.float32).reshape(())
    return out, r


def kernel(centers, features=None, labels=None, **_):
    out, _r = run(centers)
    return out


# revision 31
# speedup vs baseline: 1.0252x; 1.0252x over previous
"""Trainium2 Bass kernel for nn_DensityLoss (raw Block mode, SPMD x8, replicated).

Math
----
reference(centers, features, labels) depends only on centers [C=4096, D=256]
(features unused; labels only via N=len(labels)=262144, a constant):

    sq_i  = ||c_i||^2;  m = sum_i c_i;  S = sum sq;  proj_i = c_i . m
    n_i   = C*sq_i + S - 2*proj_i          (center_dist_i = n_i/(C-1); diag==0)
    sum n   = 2*C*S - 2*m.m                (h := m.m - 256*C*S' = -sum n/2)
    sum n^2 = C^2 q + 3C S^2 + 4 m'Sigma m - 4C (w.m) - 4S (m.m)
        q = sum sq^2, w = sum sq_i c_i, Sigma = X'X, S' = S/256
    result = k*h/d',  k = -2(C-1)^2 E/(C N),  d' = E*(sum n^2 - (sum n)^2/C)

Implementation (v3; the old kernel's GPSIMD f32->bf16 cast was its 27us
bottleneck; DMA 4MiB is the 11.1us floor):
  - DMA: p-major contiguous chunks ([4]*7+[2,2] row-tiles; small final chunks
    shorten the post-DMA critical path).
  - DVE: X cast f32->bf16 of all chunks (~0.42 cyc/elem), 14 of 32 per-row
    sq via one scalar_tensor_tensor each ((x/256)*x, accum_out = sq'=sq/256),
    per-chunk bf16 cast of the sq' column, f32 S'/q' partials (q' prescaled
    by 2^20 exactly).
  - GPSIMD: idle.  Its ~840ns/tile cast rate stalls PE's in-order tile
    consumption (paired A/B: gps=0 beats gps=1 by ~3us, gps=2 by ~5us).
  - ACT: the other 18 sq tiles via activation(Square(x/16), accum_out).
  - PE:  bf16 Gram of A=[X|1|sq'] as two full-width row blocks
    psA=G[0:128,0:258], psB=G[128:256,0:258]; tail: v0=B00 m0+B01 m1,
    v1=B10 m0+B11 m1 (4 N=1 matmuls, Gram symmetry), psS = 1'e (S',2^20 q'
    cross-partition sums), psQ = m0'[m0 w0 v0] + m1'[m1 w1 v1] = [mm, wm,
    m'Sigma m] -- so the serial scalar chain is 13 DVE ops on partition 0.
  - S'/q' stay f32 end-to-end (catastrophic var cancellation; Sigma/m/w only
    enter small terms, bf16 there is ~1e-7 on the result).
Centers replicated to all 8 cores (8-core AllReduce has a ~20us floor; the
whole kernel is shorter).

Modes: "serial" chains rounds through s_res (honest end-to-end latency per
round; kernel() runs rounds=1); "p1" chains rounds but runs the tail once
(isolates tail cost vs serial); "pipe" overlaps rounds (steady-state rate).
"""

import numpy as np

C, D = 4096, 256
N_LABELS = 262144
P = 128
NT = C // P            # 32 row tiles
W = D + 2              # 258: [X | ones | sq']
WP = 264               # padded SBUF row stride (1056 B, 32B-aligned)
GW = 260               # Ga/Gb width: [G row block (258) | pad | v col @258]
CHUNK_SIZES = [4, 4, 4, 4, 4, 4, 4, 2, 2]
ACT_TILES = [3, 2, 2, 2, 3, 2, 2, 1, 1]   # leading sq tiles per chunk on ACT
GPS_CHUNKS = 0                            # GPSIMD idle: its slow cast delayed PE's in-order consumption
DUAL_RING = True                          # odd chunks' DMA issued from the ACT HWDGE ring
N_CORES = 8

_CACHE = {}


def _build_nc(rounds=1, mode="serial"):
    import concourse.bass as bass
    from concourse import mybir

    NCH = len(CHUNK_SIZES)
    CHUNK_OFFS = [sum(CHUNK_SIZES[:i]) for i in range(NCH)]
    assert sum(CHUNK_SIZES) == NT and len(ACT_TILES) == NCH
    assert mode in ("serial", "p1", "pipe")
    serial = mode == "serial"
    f32 = mybir.dt.float32
    bf16 = mybir.dt.bfloat16
    Alu = mybir.AluOpType
    Act = mybir.ActivationFunctionType

    nc = bass.Bass()
    x_ext = nc.declare_dram_parameter("centers", [C, D], f32, isOutput=False)
    out_ext = nc.declare_dram_parameter("out", [1, 1], f32, isOutput=True)

    xv = x_ext[:, :].rearrange("(p t) d -> p t d", p=P)   # [128, 32, 256]

    from contextlib import ExitStack

    with ExitStack() as ctx:
        en = ctx.enter_context
        xh = en(nc.sbuf_tensor([P, NT, WP], f32))
        xhb = en(nc.sbuf_tensor([P, NT, WP], bf16))
        scr = en(nc.sbuf_tensor([P, D], f32))      # ACT Square main-out sink
        scrv = en(nc.sbuf_tensor([P, D], f32))     # DVE STT main-out sink
        sq2 = en(nc.sbuf_tensor([P, NT], f32))
        zc = en(nc.sbuf_tensor([P, 1], f32))
        ones_col = en(nc.sbuf_tensor([P, 1], f32))
        Ga = en(nc.sbuf_tensor([P, GW], f32))
        Gb = en(nc.sbuf_tensor([P, GW], f32))
        e = en(nc.sbuf_tensor([P, 2], f32))
        sc = en(nc.sbuf_tensor([1, 32], f32))
        res = en(nc.sbuf_tensor([1, 1], f32))
        psA = en(nc.psum_tensor([P, W], f32))
        psB = en(nc.psum_tensor([P, W], f32))
        psV0 = en(nc.psum_tensor([P, 1], f32))
        psV1 = en(nc.psum_tensor([P, 1], f32))
        psS = en(nc.psum_tensor([1, 2], f32))
        psQ = en(nc.psum_tensor([1, 3], f32))
        s_dma = [en(nc.semaphore(f"s_dma{i}")) for i in range(NCH)]
        s_pre = en(nc.semaphore("s_pre"))
        s_xb = en(nc.semaphore("s_xb"))
        s_gx = en(nc.semaphore("s_gx"))
        s_sqa = en(nc.semaphore("s_sqa"))
        s_sqd = en(nc.semaphore("s_sqd"))
        s_pe = en(nc.semaphore("s_pe"))
        s_mm = en(nc.semaphore("s_mm"))
        s_cpa = en(nc.semaphore("s_cpa"))
        s_cpb = en(nc.semaphore("s_cpb"))
        s_mv = en(nc.semaphore("s_mv"))
        s_vc = en(nc.semaphore("s_vc"))
        s_q = en(nc.semaphore("s_q"))
        s_er = en(nc.semaphore("s_er"))
        s_res = en(nc.semaphore("s_res"))
        s_out = en(nc.semaphore("s_out"))
        block = en(nc.Block())

        m0 = Ga[:, D:D + 1]
        m1 = Gb[:, D:D + 1]
        sqv = xh[:, :, D + 1]                      # [128, 32] sq' column
        n_tail = rounds if serial else 1
        Cf = float(C)
        E = 2.0 ** -20

        dual = DUAL_RING and mode != "pipe"

        @block.sync
        def _(sync):
            for r in range(rounds):
                for ci in range(NCH):
                    if dual and ci % 2 == 1:
                        continue
                    lo = CHUNK_OFFS[ci]
                    hi = lo + CHUNK_SIZES[ci]
                    if ci == 0 and r > 0:
                        if serial:
                            sync.wait_ge(s_res, r)
                        else:
                            sync.wait_ge(s_er, r)
                            sync.wait_ge(s_pe, NCH * r)
                            sync.wait_ge(s_gx, GPS_CHUNKS * r)
                    if mode == "pipe" and r > 0:
                        # WAR: round r overwrites chunk ci once round r-1's
                        # readers (DVE cast+sq'cast, ACT sq, GPS cast, PE)
                        # are done
                        sync.wait_ge(s_sqd, NCH * (r - 1) + ci + 1)
                        sync.wait_ge(s_pe, NCH * (r - 1) + ci + 1)
                        if ci < GPS_CHUNKS:
                            sync.wait_ge(s_gx, GPS_CHUNKS * (r - 1) + ci + 1)
                    sync.dma_start(
                        out=xh[:, lo:hi, 0:D], in_=xv[:, lo:hi, :],
                    ).then_inc(s_dma[ci], 16)
            sync.wait_ge(s_res, n_tail)
            sync.dma_start(out=out_ext[:, :], in_=res[:, :]).then_inc(s_out, 16)
            sync.wait_ge(s_out, 16)

        @block.vector
        def _(vector):
            # preamble (cols disjoint from DMA'd cols 0:256)
            vector.memset(xhb[:, :, D:D + 1], 1.0)
            vector.memset(zc[:, :], 0.0)
            nc.vector.memset(ones_col[:, :], 1.0).then_inc(s_pre, 1)
            TS = nc.vector.tensor_scalar
            TT = nc.vector.tensor_tensor
            STT = nc.vector.scalar_tensor_tensor
            for r in range(rounds):
                for ci in range(NCH):
                    lo = CHUNK_OFFS[ci]
                    hi = lo + CHUNK_SIZES[ci]
                    vector.wait_ge(s_dma[ci], 16 * (r + 1))
                    if ci >= GPS_CHUNKS:
                        # X cast for the whole chunk (GPS casts chunks 0-1)
                        nc.vector.tensor_copy(
                            out=xhb[:, lo:hi, 0:D], in_=xh[:, lo:hi, 0:D],
                        ).then_inc(s_xb, 1)
                    # sq via (x/256)*x with accum -> sq' = sq/256 (f32)
                    for t in range(lo + ACT_TILES[ci], hi):
                        STT(scrv[:, :], xh[:, t, 0:D], 1.0 / 256.0,
                            xh[:, t, 0:D], op0=Alu.mult, op1=Alu.mult,
                            accum_out=xh[:, t, D + 1:D + 2])
                    vector.wait_ge(s_sqa, NCH * r + ci + 1)
                    vector.drain()
                    nc.vector.tensor_copy(
                        out=xhb[:, lo:hi, D + 1:D + 2],
                        in_=xh[:, lo:hi, D + 1:D + 2],
                    ).then_inc(s_sqd, 1)
                # S' and 2^20*q' partials (f32, precision-critical; 2^20 exact)
                nc.vector.tensor_reduce(e[:, 0:1], sqv,
                                        axis=mybir.AxisListType.X, op=Alu.add)
                STT(sq2[:, :], sqv, 2.0 ** 20, sqv, op0=Alu.mult, op1=Alu.mult)
                vector.drain()
                nc.vector.tensor_reduce(e[:, 1:2], sq2[:, :],
                                        axis=mybir.AxisListType.X,
                                        op=Alu.add).then_inc(s_er, 1)
                if not serial and r < rounds - 1:
                    continue
                # ---- tail ----
                rr = (r + 1) if serial else 1
                vector.wait_ge(s_mm, rr)
                nc.vector.tensor_copy(Gb[:, 0:W], psB[:, :]).then_inc(s_cpb, 1)
                vector.wait_ge(s_mv, rr)
                nc.vector.tensor_copy(Ga[:, W:W + 1], psV0[:, :])
                nc.vector.tensor_copy(Gb[:, W:W + 1],
                                      psV1[:, :]).then_inc(s_vc, 1)
                vector.wait_ge(s_q, rr)
                # scalar chain on partition 0.  h^2 in T6 is expanded exactly
                # (h = mm - 2^20 S'; pow2/integer coefficients), merging into
                # T2,T5: every d' term is a direct product of stage-0 values:
                #   d' = T1 - 256 S'^2 + 4E mSm - 4 wm + 2^-10 S' mm
                #        - 2^-30 mm^2
                # sc: 0 mm | 1 wm | 2 mSm | 18 S' | 19 T1=2^20 q' | 8 h
                #     | 20..24 T-terms | 11 d' | 12 1/d'

                def s(i):
                    return sc[:, i:i + 1]

                nc.vector.tensor_copy(sc[:, 0:3], psQ[0:1, 0:3])
                nc.vector.tensor_copy(sc[:, 18:20], psS[0:1, 0:2])
                vector.drain()
                STT(s(20), s(18), -256.0, s(18), op0=Alu.mult, op1=Alu.mult)
                STT(s(21), s(18), 2.0 ** -10, s(0), op0=Alu.mult,
                    op1=Alu.mult)
                STT(s(22), s(0), -(2.0 ** -30), s(0), op0=Alu.mult,
                    op1=Alu.mult)
                TS(s(23), s(2), 4.0 * E, None, op0=Alu.mult)
                TS(s(24), s(1), -4.0, None, op0=Alu.mult)
                STT(s(8), s(18), -256.0 * Cf, s(0), op0=Alu.mult, op1=Alu.add)
                vector.drain()
                k = -2.0 * (Cf - 1.0) ** 2 / (Cf * float(N_LABELS)) * E
                nc.vector.tensor_reduce(s(11), sc[:, 19:25],
                                        axis=mybir.AxisListType.X, op=Alu.add)
                TS(s(9), s(8), k, None, op0=Alu.mult)          # h*k
                vector.drain()
                nc.vector.reciprocal(s(12), s(11))
                vector.drain()
                nc.vector.tensor_mul(res[:, :], s(9),
                                     s(12)).then_inc(s_res, 1)

        @block.scalar
        def _(scalar):
            scalar.wait_ge(s_pre, 1)
            for r in range(rounds):
                if dual:
                    # odd chunks' DMA from the second HWDGE ring, issued
                    # before this round's squares
                    for ci in range(1, NCH, 2):
                        lo = CHUNK_OFFS[ci]
                        hi = lo + CHUNK_SIZES[ci]
                        if ci == 1 and r > 0:
                            if serial:
                                scalar.wait_ge(s_res, r)
                            else:
                                scalar.wait_ge(s_er, r)
                                scalar.wait_ge(s_pe, NCH * r)
                        nc.scalar.dma_start(
                            out=xh[:, lo:hi, 0:D], in_=xv[:, lo:hi, :],
                        ).then_inc(s_dma[ci], 16)
                if mode == "pipe" and r > 0:
                    # WAR: round r's accum writes to the sq' col race round
                    # r-1's e reductions on DVE
                    scalar.wait_ge(s_er, r)
                for ci in range(NCH):
                    lo = CHUNK_OFFS[ci]
                    if ACT_TILES[ci] == 0:
                        # keep s_sqa chunk-ordered; fires early, off the path
                        nc.scalar.copy(out=scr[:, 0:1],
                                       in_=zc[:, :]).then_inc(s_sqa, 1)
                        continue
                    scalar.wait_ge(s_dma[ci], 16 * (r + 1))
                    op = None
                    for t in range(lo, lo + ACT_TILES[ci]):
                        op = nc.scalar.activation(
                            out=scr[:, :], in_=xh[:, t, 0:D], func=Act.Square,
                            bias=zc[:, :], scale=0.0625,
                            accum_out=xh[:, t, D + 1:D + 2])
                    op.then_inc(s_sqa, 1)
                if serial or r == rounds - 1:
                    scalar.wait_ge(s_mm, (r + 1) if serial else 1)
                    nc.scalar.copy(Ga[:, 0:W], psA[:, :]).then_inc(s_cpa, 1)

        @block.gpsimd
        def _(gpsimd):
            for r in range(rounds):
                for ci in range(GPS_CHUNKS):
                    lo = CHUNK_OFFS[ci]
                    hi = lo + CHUNK_SIZES[ci]
                    gpsimd.wait_ge(s_dma[ci], 16 * (r + 1))
                    nc.gpsimd.tensor_copy(
                        out=xhb[:, lo:hi, 0:D], in_=xh[:, lo:hi, 0:D],
                    ).then_inc(s_gx, 1)

        @block.tensor
        def _(tensor):
            tensor.wait_ge(s_pre, 1)
            for r in range(rounds):
                for t in range(NT):
                    ci = next(i for i in range(NCH)
                              if CHUNK_OFFS[i] <= t < CHUNK_OFFS[i]
                              + CHUNK_SIZES[i])
                    if ci < GPS_CHUNKS:
                        tensor.wait_ge(s_gx, GPS_CHUNKS * r + ci + 1)
                    else:
                        tensor.wait_ge(s_xb,
                                       (NCH - GPS_CHUNKS) * r
                                       + ci - GPS_CHUNKS + 1)
                    tensor.wait_ge(s_sqd, NCH * r + ci + 1)
                    first = (t == 0 and (serial or r == 0))
                    last = (t == NT - 1 and (serial or r == rounds - 1))
                    ma = nc.tensor.matmul(psA[:, :], xhb[:, t, 0:P],
                                          xhb[:, t, 0:W], start=first,
                                          stop=last)
                    mm = nc.tensor.matmul(psB[:, :], xhb[:, t, P:D],
                                          xhb[:, t, 0:W], start=first,
                                          stop=last)
                    if t == CHUNK_OFFS[ci] + CHUNK_SIZES[ci] - 1:
                        ma.then_inc(s_pe, 1)
                    if last:
                        mm.then_inc(s_mm, 1)
                if serial or r == rounds - 1:
                    rr = (r + 1) if serial else 1
                    tensor.wait_ge(s_er, rr)
                    nc.tensor.matmul(psS[:, :], ones_col[:, :], e[:, :],
                                     start=True, stop=True)
                    tensor.wait_ge(s_cpa, rr)
                    tensor.wait_ge(s_cpb, rr)
                    # v0 = B00 m0 + B01 m1; v1 = B10 m0 + B11 m1 (symmetry)
                    nc.tensor.matmul(psV0[:, :], Ga[:, 0:P], m0,
                                     start=True, stop=False)
                    nc.tensor.matmul(psV0[:, :], Gb[:, 0:P], m1,
                                     start=False, stop=True)
                    nc.tensor.matmul(psV1[:, :], Ga[:, P:D], m0,
                                     start=True, stop=False)
                    nc.tensor.matmul(psV1[:, :], Gb[:, P:D], m1,
                                     start=False, stop=True).then_inc(s_mv, 1)
                    tensor.wait_ge(s_vc, rr)
                    # psQ = m0'[m0 w0 v0] + m1'[m1 w1 v1] = [mm, wm, mSm]
                    nc.tensor.matmul(psQ[:, :], m0, Ga[:, D:D + 3],
                                     start=True, stop=False)
                    nc.tensor.matmul(psQ[:, :], m1, Gb[:, D:D + 3],
                                     start=False, stop=True).then_inc(s_q, 1)

    return nc


def _get_nc(rounds=1, mode="serial"):
    key = ("nc", rounds, mode)
    if key not in _CACHE:
        _CACHE[key] = _build_nc(rounds, mode)
    return _CACHE[key]


def run(centers: np.ndarray, trace: bool = False):
    """Run the SPMD kernel on cores 0-7; returns (scalar ndarray, results)."""
    from concourse.bass_utils import run_bass_kernel_spmd

    nc = _get_nc()
    x = np.ascontiguousarray(np.asarray(centers, dtype=np.float32))
    in_maps = [{"centers": x} for _ in range(N_CORES)]
    r = run_bass_kernel_spmd(nc, in_maps, core_ids=list(range(N_CORES)),
                             trace=trace)
    # all 8 cores compute the same scalar; median guards a flaky core
    vals = np.array([np.asarray(res["out"]).reshape(()) for res in r.results],
                    dtype=np.float32)
    out = np.median(vals).astype(np.float32).reshape(())
    return out, r


def kernel(centers, features=None, labels=None, **_):
    out, _r = run(centers)
    return out
